# revision 9
# baseline (speedup 1.0000x reference)
"""Trainium2 Bass kernel for nn_Criterion_37984690765901.

Loss =  L_t + lam_e * Loss_e + lam_od * (L_zt + L_zs)
  L_t    = mean_r( lse(y_zt_r) - y_zt[r, target_r] )            (cross entropy)
  Loss_e = mean_r( lse(s_r) - (sum_j e^{s_rj} s_rj)/sum_j e^{s_rj} )   (entropy)
  L_zt/L_zs = mean_r( rowdot_r/S_r - ln S_r + ln PS_r )          (KLD batchmean)
     with enc = mean + exp(0.5*log_std)*eps,  e = exp(enc), S = sum_d e,
     pe = exp(prior), PS = sum_d pe, rowdot = sum_d e*(enc - prior).
     (prior_s = 1 + eps_prior_s, but KLD is shift-invariant in the prior
      logits, so eps_prior_s is used directly.)

v4 hybrid design (baseline f32 was 151.6us, v2 all-PE-reduce was 75.5us):
  * All big tensors shipped bf16 (halves HBM traffic; host sim of the full
    quantization chain incl. bf16 pairwise-tree sums: rel err 2.9e-5 vs the
    2e-2 gate). log_std pre-halved on host so [0.5*ls | prior] exps in 1 ACT.
  * The elementwise chain (se, enc, e, d, ed) is layout-agnostic and
    identical for both branches: DVE tensor_tensor at 2x (16-bit) + ACT exp.
  * The per-row sum_d reductions are the scarce resource; no single engine
    can hold all six branch-stats, so the two branches use different
    layouts to split them across engines:
      - branch t: rows on partitions, D contiguous on free axis. S+RD
        reduce as ONE in-place bf16 pairwise-tree (7 TT levels at 2x) on
        DVE over the adjacent [e | ed] tile; the PS tree runs on GPSIMD.
      - branch s: D=128 on partitions, rows on free axis. All three sums
        are TensorEngine data-as-stationary x ones[128,1] matmuls into
        PSUM columns (192 LDW+MM pairs ~ 27us, PE is otherwise idle).
  * Engines/step target: DMA 5.9us, ACT 5.5us, DVE ~5.5us, GPSIMD ~2.9us
    (t-steps), PE ~6.7us (s-steps).

Sharding: pure data parallel over the batch axis, 8192 rows per core; the
batch reduction finishes on the host in float64.

Device per-core outputs: out[128, 256] f32 =
  [:, 0:64]    per-row KL contribution, t branch   (row = 64p + j)
  [:, 64:128]  per-row KL contribution, s branch   (row = col*128 + p)
  [:, 128:192] per-row (lse_y - y_pick)            (row = 64p + j)
  [:, 192:256] per-row entropy of softmax(s_zt)
Host combine just sums everything in f64, so orderings don't matter.
"""

import os
import numpy as np

NCORES = 8
B, D, C, S = 65536, 128, 10, 2
LAMBDA_E, LAMBDA_OD = 0.1, 0.036
GAMMA_E, GAMMA_OD = 2.0, 2.0
STEP_SIZE = 1000.0

RPC = B // NCORES            # rows per core = 8192
P = 128                      # SBUF partitions
CHUNK = 2048                 # free elems per chunk (= R rows for branch s)
G = CHUNK // D               # 16 row-groups per chunk (branch t)
NCH = RPC * D // P // CHUNK  # 4 chunks per tensor
NBLK = CHUNK // 128          # 16 row-blocks (matmuls) per chunk (branch s)
NCOL = 64                    # stat columns per branch
YF = RPC * C // P            # 640
SF = RPC * S // P            # 128

BRANCHES = ["bt", "bs"]

_CACHED_NC = None
LAST_EXEC_NS = None


def _build_nc():
    import concourse.bass as bass
    import concourse.tile as tile
    from concourse import mybir
    from contextlib import ExitStack

    f32 = mybir.dt.float32
    bf16 = mybir.dt.bfloat16
    Exp = mybir.ActivationFunctionType.Exp
    Ln = mybir.ActivationFunctionType.Ln
    add = mybir.AluOpType.add
    sub = mybir.AluOpType.subtract
    mult = mybir.AluOpType.mult
    X = mybir.AxisListType.X

    nc = bass.Bass("TRN2", debug=False)

    ins = {}
    for bn in BRANCHES:
        ins[bn] = nc.dram_tensor(
            bn, [P, NCH, 4 * CHUNK], bf16, kind="ExternalInput"
        ).ap()
    ins["yoh"] = nc.dram_tensor("yoh", [P, 2 * YF], bf16, kind="ExternalInput").ap()
    ins["sz"] = nc.dram_tensor("sz", [P, SF], bf16, kind="ExternalInput").ap()
    out_d = nc.dram_tensor("out", [P, 4 * NCOL], f32, kind="ExternalOutput").ap()

    with tile.TileContext(nc) as tc, ExitStack() as ctx:
        io = ctx.enter_context(tc.tile_pool(name="io", bufs=5))
        pep = ctx.enter_context(tc.tile_pool(name="pep", bufs=4))
        eep = ctx.enter_context(tc.tile_pool(name="eep", bufs=4))
        st = ctx.enter_context(tc.tile_pool(name="st", bufs=1))
        ps = ctx.enter_context(tc.psum_pool(name="ps", bufs=1))

        out_sb = st.tile([P, 4 * NCOL], f32, tag="out")

        # branch t stats (SBUF): SRD[:,0,:] = sum(e), SRD[:,1,:] = sum(e*d)
        SRDt = st.tile([P, 2, NCOL], f32, tag="SRDt")
        PSt = st.tile([P, NCOL], f32, tag="PSt")
        # branch s stats (PSUM): column col = chunk*NBLK + blk = rows
        # col*128..col*128+127
        S_s = ps.tile([P, NCOL], f32, tag="S_s", name="S_s")
        RD_s = ps.tile([P, NCOL], f32, tag="RD_s", name="RD_s")
        PS_s = ps.tile([P, NCOL], f32, tag="PS_s", name="PS_s")

        ones_t = st.tile([P, 1], bf16, tag="ones")
        nc.vector.memset(ones_t[:], 1.0)

        NSTEPS = 2 * NCH
        state = {}

        def stage0(s):
            # DMA chunk s: [0.5*ls | prior] first (feeds ACT soonest)
            b, c = s % 2, s // 2
            t = io.tile([P, 4 * CHUNK], bf16, tag="pk", name=f"pk{s}")
            nc.sync.dma_start(
                t[:, 0:2 * CHUNK], ins[BRANCHES[b]][:, c, 0:2 * CHUNK])
            nc.sync.dma_start(
                t[:, 2 * CHUNK:4 * CHUNK],
                ins[BRANCHES[b]][:, c, 2 * CHUNK:4 * CHUNK])
            state[s] = t

        def stage1(s):
            # ACT: [std | pe] = exp([0.5*ls | prior]) in one instruction
            t = state[s]
            sp = pep.tile([P, 2 * CHUNK], bf16, tag="sp", name=f"sp{s}")
            nc.scalar.activation(sp[:], t[:, 0:2 * CHUNK], Exp)
            state[(s, "sp")] = sp

        def stage2(s):
            # DVE se/enc (2x bf16), ACT e, DVE d — branch-agnostic
            t = state[s]
            sp = state[(s, "sp")]
            # se = std * eps            (into eps slot)
            nc.vector.tensor_tensor(
                t[:, 2 * CHUNK:3 * CHUNK], sp[:, 0:CHUNK],
                t[:, 2 * CHUNK:3 * CHUNK], mult)
            # enc = se + mean           (into mean slot)
            nc.vector.tensor_tensor(
                t[:, 3 * CHUNK:4 * CHUNK], t[:, 2 * CHUNK:3 * CHUNK],
                t[:, 3 * CHUNK:4 * CHUNK], add)
            ee = eep.tile([P, 2 * CHUNK], bf16, tag="ee", name=f"ee{s}")
            # e = exp(enc)
            nc.scalar.activation(ee[:, 0:CHUNK], t[:, 3 * CHUNK:4 * CHUNK], Exp)
            # d = enc - prior           (into dead ls slot; overlaps ACT e)
            nc.vector.tensor_tensor(
                t[:, 0:CHUNK], t[:, 3 * CHUNK:4 * CHUNK],
                t[:, CHUNK:2 * CHUNK], sub)
            state[(s, "ee")] = ee

        def stage2b(s):
            # ed = e * d, one round later so the in-order DVE stream never
            # blocks on the ACT queue (e(s) sits behind sp(s+1) there)
            t = state.pop(s)
            ee = state[(s, "ee")]
            nc.vector.tensor_tensor(
                ee[:, CHUNK:2 * CHUNK], ee[:, 0:CHUNK], t[:, 0:CHUNK], mult)

        def stage_ps_t(s):
            # GPSIMD: branch-t PS chunk reduce as an in-place bf16 pairwise
            # tree over the pe half of sp, final f32 level into PSt
            c = s // 2
            sp = state[(s, "sp")]
            v = sp[:, CHUNK:2 * CHUNK].rearrange("p (g d) -> p g d", d=D)
            w = D // 2
            while w >= 2:
                nc.gpsimd.tensor_tensor(
                    v[:, :, 0:w], v[:, :, 0:w], v[:, :, w:2 * w], add)
                w //= 2
            nc.gpsimd.tensor_tensor(
                PSt[:, bass.ts(c, G)].rearrange("p (g o) -> p g o", o=1),
                v[:, :, 0:1], v[:, :, 1:2], add)

        def stage_srd_t(s):
            # DVE: branch-t combined [e | ed] in-place bf16 pairwise tree
            # (7 levels at 2x), final f32 level straight into SRDt
            c = s // 2
            ee = state.pop((s, "ee"))
            state.pop((s, "sp"))
            v = ee[:].rearrange("p (k g d) -> p k g d", k=2, d=D)
            w = D // 2
            while w >= 2:
                nc.vector.tensor_tensor(
                    v[:, :, :, 0:w], v[:, :, :, 0:w], v[:, :, :, w:2 * w], add)
                w //= 2
            nc.vector.tensor_tensor(
                SRDt[:, :, bass.ts(c, G)].rearrange(
                    "p k (g o) -> p k g o", o=1),
                v[:, :, :, 0:1], v[:, :, :, 1:2], add)

        def stage_pe_s(s):
            # PE: branch-s 48 matmuls; each reduces a [128 D, 128 rows]
            # block against ones -> one PSUM column of per-row sums.
            # Emitted in readiness order (pe, then e, then ed).
            c = s // 2
            sp = state.pop((s, "sp"))
            ee = state.pop((s, "ee"))
            for j in range(NBLK):
                col = c * NBLK + j
                nc.tensor.matmul(
                    PS_s[:, col:col + 1],
                    sp[:, CHUNK + 128 * j:CHUNK + 128 * (j + 1)], ones_t[:],
                    start=True, stop=True)
            for j in range(NBLK):
                col = c * NBLK + j
                nc.tensor.matmul(
                    S_s[:, col:col + 1],
                    ee[:, 128 * j:128 * (j + 1)], ones_t[:],
                    start=True, stop=True)
            for j in range(NBLK):
                col = c * NBLK + j
                nc.tensor.matmul(
                    RD_s[:, col:col + 1],
                    ee[:, CHUNK + 128 * j:CHUNK + 128 * (j + 1)], ones_t[:],
                    start=True, stop=True)

        # --- small blocks first: their DMA + compute fill the pipeline
        # warm-up while the first big chunks stream in ---
        # cross entropy on y_zt: per-row lse - picked
        yoh_t = st.tile([P, 2 * YF], bf16, tag="yoh")
        nc.sync.dma_start(yoh_t[:], ins["yoh"][:])
        y_ap = yoh_t[:, 0:YF]
        oh_ap = yoh_t[:, YF:2 * YF]
        ey_t = st.tile([P, YF], bf16, tag="ey")
        nc.scalar.activation(ey_t[:], y_ap, Exp)
        sy_t = st.tile([P, NCOL], f32, tag="sy")
        nc.vector.tensor_reduce(
            sy_t[:], ey_t[:].rearrange("p (g c) -> p g c", c=C), X, add
        )
        lse_t = st.tile([P, NCOL], f32, tag="lse")
        nc.scalar.activation(lse_t[:], sy_t[:], Ln)
        ym_t = st.tile([P, YF], bf16, tag="ym")
        nc.vector.tensor_tensor(ym_t[:], y_ap, oh_ap, mult)
        pick_t = st.tile([P, NCOL], f32, tag="pick")
        nc.vector.tensor_reduce(
            pick_t[:], ym_t[:].rearrange("p (g c) -> p g c", c=C), X, add
        )
        nc.vector.tensor_tensor(
            out_sb[:, 2 * NCOL:3 * NCOL], lse_t[:], pick_t[:], sub
        )
        nc.sync.dma_start(out_d[:, 2 * NCOL:3 * NCOL], out_sb[:, 2 * NCOL:3 * NCOL])

        # entropy of softmax(s_zt): per-row lse - (sum e*x)/s
        sz_t = st.tile([P, SF], bf16, tag="sz")
        nc.sync.dma_start(sz_t[:], ins["sz"][:])
        esz_t = st.tile([P, SF], bf16, tag="esz")
        nc.scalar.activation(esz_t[:], sz_t[:], Exp)
        ssum_t = st.tile([P, NCOL], f32, tag="ssum")
        nc.vector.tensor_reduce(
            ssum_t[:], esz_t[:].rearrange("p (g c) -> p g c", c=S), X, add
        )
        exs_t = st.tile([P, SF], bf16, tag="exs")
        nc.vector.tensor_tensor(exs_t[:], esz_t[:], sz_t[:], mult)
        dsum_t = st.tile([P, NCOL], f32, tag="dsum")
        nc.vector.tensor_reduce(
            dsum_t[:], exs_t[:].rearrange("p (g c) -> p g c", c=S), X, add
        )
        rss_t = st.tile([P, NCOL], f32, tag="rss")
        nc.vector.reciprocal(rss_t[:], ssum_t[:])
        t2_t = st.tile([P, NCOL], f32, tag="t2")
        nc.vector.tensor_tensor(t2_t[:], dsum_t[:], rss_t[:], mult)
        lss_t = st.tile([P, NCOL], f32, tag="lss")
        nc.scalar.activation(lss_t[:], ssum_t[:], Ln)
        nc.vector.tensor_tensor(
            out_sb[:, 3 * NCOL:4 * NCOL], lss_t[:], t2_t[:], sub
        )
        nc.sync.dma_start(out_d[:, 3 * NCOL:4 * NCOL], out_sb[:, 3 * NCOL:4 * NCOL])

        # --- big-branch software pipeline ---
        # lags: DMA(i), sp(i-1), se/enc/e/d(i-2), ed(i-3),
        #       PE/gpsimd(i-3), DVE tree(i-4)
        for i in range(NSTEPS + 4):
            if i < NSTEPS:
                stage0(i)
            if 1 <= i and i - 1 < NSTEPS:
                stage1(i - 1)
            if 2 <= i and i - 2 < NSTEPS:
                stage2(i - 2)
            if 3 <= i and i - 3 < NSTEPS:
                s = i - 3
                stage2b(s)
                if s % 2 == 0:
                    stage_ps_t(s)
                else:
                    stage_pe_s(s)
            if 4 <= i and i - 4 < NSTEPS:
                s = i - 4
                if s % 2 == 0:
                    stage_srd_t(s)

        # tails: kl_row = RD/S - ln S + ln PS  (once per branch)
        for b, (S_ap, RD_ap, PS_ap) in enumerate(
            [(SRDt[:, 0, :], SRDt[:, 1, :], PSt[:]),
             (S_s[:], RD_s[:], PS_s[:])]
        ):
            rs_t = st.tile([P, NCOL], f32, tag=f"rs{b}", name=f"rs{b}")
            nc.vector.reciprocal(rs_t[:], S_ap)
            term_t = st.tile([P, NCOL], f32, tag=f"term{b}", name=f"term{b}")
            nc.vector.tensor_tensor(term_t[:], RD_ap, rs_t[:], mult)
            lnS_t = st.tile([P, NCOL], f32, tag=f"lnS{b}", name=f"lnS{b}")
            nc.scalar.activation(lnS_t[:], S_ap, Ln)
            lnPS_t = st.tile([P, NCOL], f32, tag=f"lnPS{b}", name=f"lnPS{b}")
            nc.scalar.activation(lnPS_t[:], PS_ap, Ln)
            tmp_t = st.tile([P, NCOL], f32, tag=f"tmp{b}", name=f"tmp{b}")
            nc.vector.tensor_tensor(tmp_t[:], term_t[:], lnS_t[:], sub)
            nc.vector.tensor_tensor(
                out_sb[:, b * NCOL:(b + 1) * NCOL], tmp_t[:], lnPS_t[:], add
            )
            nc.sync.dma_start(
                out_d[:, b * NCOL:(b + 1) * NCOL],
                out_sb[:, b * NCOL:(b + 1) * NCOL]
            )

    return nc


def _split_multi_waits(nc):
    """walrus's codegen allows a single embedded sync-wait per compute
    instruction; Tile sometimes emits two (e.g. ACT + DMA deps on one TT).
    Hoist all-but-one wait into standalone EventSemaphore instructions
    placed immediately before, on the same engine. Applied at BIR-JSON
    serialization time so CoreSim (which handles multi-wait fine) is
    untouched."""
    import json

    orig = nc.to_json_bytes

    def patched():
        bj = json.loads(orig())
        for fn in bj["functions"]:
            for blk in fn["blocks"]:
                new = []
                for inst in blk["instructions"]:
                    si = inst.get("sync_info") or {}
                    waits = si.get("on_wait") or []
                    if len(waits) > 1 and inst.get("opcode") != "EventSemaphore":
                        for i, w in enumerate(waits[:-1]):
                            new.append({
                                "debug": inst.get("debug"),
                                "engine": inst["engine"],
                                "ins": [],
                                "name": f"{inst['name']}-sw{i}",
                                "opcode": "EventSemaphore",
                                "outs": [],
                                "sync_info": {"on_update": [], "on_wait": [w]},
                            })
                        si["on_wait"] = [waits[-1]]
                    new.append(inst)
                blk["instructions"] = new
        return json.dumps(bj).encode()

    nc.to_json_bytes = patched
    return nc


def get_nc():
    global _CACHED_NC
    if _CACHED_NC is None:
        _CACHED_NC = _split_multi_waits(_build_nc())
    return _CACHED_NC


def make_in_maps(inputs):
    """Shard the full inputs into per-core in_maps for run_bass_kernel_spmd.

    branch t ("bt"): rows on partitions — [8192, 128] -> [P, NCH, CHUNK]
    branch s ("bs"): D on partitions — [8192, 128].T -> [P, NCH, CHUNK]
    both packed as [P, NCH, 4, CHUNK] with slices [0.5*ls | prior | eps | mean]
    """
    import ml_dtypes
    bf16 = ml_dtypes.bfloat16
    f32 = np.float32
    arr = {k: np.asarray(v) for k, v in inputs.items()}
    target = np.asarray(arr["target"]).astype(np.int64).reshape(B)
    onehot = np.zeros((B, C), dtype=f32)
    onehot[np.arange(B), target] = 1.0

    big = {}
    for bn, srcs in (("bt", ("log_std_t", "eps_prior_t", "eps_t", "mean_t")),
                     ("bs", ("log_std_s", "eps_prior_s", "eps_s", "mean_s"))):
        mats = []
        for i, s in enumerate(srcs):
            a = np.asarray(arr[s], dtype=f32)
            if i == 0:
                a = a * 0.5
            mats.append(a.astype(bf16))
        big[bn] = mats

    in_maps = []
    for cidx in range(NCORES):
        sl = slice(cidx * RPC, (cidx + 1) * RPC)
        m = {}
        pk = np.stack(
            [a[sl].reshape(P, NCH, CHUNK) for a in big["bt"]], axis=2)
        m["bt"] = np.ascontiguousarray(pk.reshape(P, NCH, 4 * CHUNK))
        pk = np.stack(
            [np.ascontiguousarray(a[sl].T).reshape(P, NCH, CHUNK)
             for a in big["bs"]], axis=2)
        m["bs"] = np.ascontiguousarray(pk.reshape(P, NCH, 4 * CHUNK))
        yoh = np.empty((P, 2 * YF), dtype=bf16)
        yoh[:, :YF] = np.ascontiguousarray(arr["y_zt"][sl], dtype=f32).reshape(P, YF).astype(bf16)
        yoh[:, YF:] = np.ascontiguousarray(onehot[sl]).reshape(P, YF).astype(bf16)
        m["yoh"] = yoh
        m["sz"] = np.ascontiguousarray(arr["s_zt"][sl], dtype=f32).reshape(P, SF).astype(bf16)
        in_maps.append(m)
    return in_maps


def combine(outs, current_step):
    """Host-side unshard: f64 reduce of per-row partials -> final f32 scalar."""
    tot = np.zeros(4, dtype=np.float64)
    for o in outs:
        o = o.reshape(P, 4, NCOL)
        tot += o.sum(axis=(0, 2), dtype=np.float64)
    L_zt, L_zs, L_t, Loss_e = tot / B
    frac = float(current_step) / STEP_SIZE
    lam_e = LAMBDA_E * GAMMA_E ** frac
    lam_od = LAMBDA_OD * GAMMA_OD ** frac
    val = L_t + lam_e * Loss_e + lam_od * (L_zt + L_zs)
    return np.array(val, dtype=np.float32)


def _install_ntff_hook():
    """Best-effort: register the axon NTFF profiling hook that the agent
    image's antenv package is missing, so trace=True yields exec_time_ns."""
    try:
        import sys, types
        import antenv
        if "antenv.axon_hooks" in sys.modules:
            return True
        sys.path.insert(0, "/root/.axon_site/trn_agent_boot")
        import trn_boot
        mod = types.ModuleType("antenv.axon_hooks")
        _h = {}
        mod.set_axon_ntff_profile_hook = lambda h: _h.__setitem__("h", h)
        mod.get_axon_ntff_profile_hook = lambda: _h.get("h")
        sys.modules["antenv.axon_hooks"] = mod
        antenv.axon_hooks = mod
        mod.set_axon_ntff_profile_hook(
            trn_boot._ntff_profile_via_ctypes("/opt/axon/libaxon_pjrt.so")
        )
        import concourse.bass_utils as bu
        bu.upload_artifacts = lambda tmpdir: str(tmpdir)
        return True
    except Exception:
        return False


def kernel(**inputs):
    global LAST_EXEC_NS
    from concourse.bass_utils import run_bass_kernel_spmd

    trace = os.environ.get("BASS_KERNEL_TRACE", "0") == "1"
    if trace:
        trace = _install_ntff_hook()

    nc = get_nc()
    in_maps = make_in_maps(inputs)
    res = run_bass_kernel_spmd(
        nc, in_maps, list(range(NCORES)), trace=trace
    )
    LAST_EXEC_NS = res.exec_time_ns
    outs = [r["out"] for r in res.results]
    cs = inputs.get("current_step", 500)
    return combine(outs, int(np.asarray(cs)))


# revision 10
# speedup vs baseline: 1.2018x; 1.2018x over previous
"""Trainium2 Bass kernel for nn_Criterion_37984690765901.

Loss =  L_t + lam_e * Loss_e + lam_od * (L_zt + L_zs)
  L_t    = mean_r( lse(y_zt_r) - y_zt[r, target_r] )            (cross entropy)
  Loss_e = mean_r( lse(s_r) - (sum_j e^{s_rj} s_rj)/sum_j e^{s_rj} )   (entropy)
  L_zt/L_zs = mean_r( rowdot_r/S_r - ln S_r + ln PS_r )          (KLD batchmean)
     with enc = mean + exp(0.5*log_std)*eps,  e = exp(enc), S = sum_d e,
     pe = exp(prior), PS = sum_d pe, rowdot = sum_d e*(enc - prior).
     (prior_s = 1 + eps_prior_s, but KLD is shift-invariant in the prior
      logits, so eps_prior_s is used directly.)

v5 design (baseline f32 151.6us; v2 all-PE 75.5us; hybrid DVE-tree 93.8us):
  * All big tensors shipped bf16 (halves HBM traffic; host sim of the full
    quantization chain gives rel err ~4e-6 vs the 2e-2 gate). log_std is
    pre-halved on the host so [0.5*ls | prior] exps in one ACT op.
  * D=128 on partitions, rows on the free axis. Elementwise chain on DVE at
    2x (16-bit packed) + ACT exp; all six per-row sum_d reductions are
    TensorEngine data-as-stationary x ones[128,1] matmuls into PSUM columns.
    Measured: one [128x128] block costs ~140ns effective (LDW overlaps MM),
    giving a dense ~54us PE stream — the critical path, so everything else
    is arranged to keep PE fed from t~8us with zero stalls:
      - PS matmuls (need only sp = exp([ls'|prior])) run at pipeline lag 2,
        S/RD matmuls at lag 3, so the PE stream starts as soon as the first
        chunk's exps land and never waits mid-stream (DVE+ACT per round are
        faster than PE's 6.7us).
      - the small y_zt/s_zt blocks are emitted after the first big round so
        they don't delay sp(0) on the in-order ACT queue.
      - ed lags one round on DVE so the in-order DVE stream never blocks on
        the ACT queue (e(s) sits behind sp(s+1) there).

Sharding: pure data parallel over the batch axis, 8192 rows per core; the
batch reduction finishes on the host in float64.

Device per-core outputs: out[128, 256] f32 =
  [:, 0:64]    per-row KL contribution, t branch   (row = col*128 + p)
  [:, 64:128]  per-row KL contribution, s branch   (row = col*128 + p)
  [:, 128:192] per-row (lse_y - y_pick)            (row = 64p + j)
  [:, 192:256] per-row entropy of softmax(s_zt)
Host combine just sums everything in f64, so orderings don't matter.
"""

import os
import numpy as np

NCORES = 8
B, D, C, S = 65536, 128, 10, 2
LAMBDA_E, LAMBDA_OD = 0.1, 0.036
GAMMA_E, GAMMA_OD = 2.0, 2.0
STEP_SIZE = 1000.0

RPC = B // NCORES            # rows per core = 8192
P = 128                      # SBUF partitions
CHUNK = 2048                 # rows per chunk
NCH = RPC // CHUNK           # 4 chunks per tensor
NBLK = CHUNK // 128          # 16 row-blocks (matmuls) per chunk per stat
NCOL = 64                    # stat columns per branch (8192/128)
YF = RPC * C // P            # 640
SF = RPC * S // P            # 128

BRANCHES = ["bt", "bs"]

_CACHED_NC = None
LAST_EXEC_NS = None


def _build_nc():
    import concourse.bass as bass
    import concourse.tile as tile
    from concourse import mybir
    from contextlib import ExitStack

    f32 = mybir.dt.float32
    bf16 = mybir.dt.bfloat16
    Exp = mybir.ActivationFunctionType.Exp
    Ln = mybir.ActivationFunctionType.Ln
    add = mybir.AluOpType.add
    sub = mybir.AluOpType.subtract
    mult = mybir.AluOpType.mult
    X = mybir.AxisListType.X

    nc = bass.Bass("TRN2", debug=False)

    ins = {}
    for bn in BRANCHES:
        ins[bn] = nc.dram_tensor(
            bn, [P, NCH, 4 * CHUNK], bf16, kind="ExternalInput"
        ).ap()
    ins["yoh"] = nc.dram_tensor("yoh", [P, 2 * YF], bf16, kind="ExternalInput").ap()
    ins["sz"] = nc.dram_tensor("sz", [P, SF], bf16, kind="ExternalInput").ap()
    out_d = nc.dram_tensor("out", [P, 4 * NCOL], f32, kind="ExternalOutput").ap()

    with tile.TileContext(nc) as tc, ExitStack() as ctx:
        io = ctx.enter_context(tc.tile_pool(name="io", bufs=5))
        pep = ctx.enter_context(tc.tile_pool(name="pep", bufs=4))
        eep = ctx.enter_context(tc.tile_pool(name="eep", bufs=4))
        st = ctx.enter_context(tc.tile_pool(name="st", bufs=1))
        ps = ctx.enter_context(tc.psum_pool(name="ps", bufs=1))

        out_sb = st.tile([P, 4 * NCOL], f32, tag="out")

        # PSUM stat tiles: column col = chunk*NBLK + blk holds rows
        # col*128 .. col*128+127 of this core's branch shard.
        S_ps = [ps.tile([P, NCOL], f32, tag=f"S{b}", name=f"S{b}")
                for b in range(2)]
        RD_ps = [ps.tile([P, NCOL], f32, tag=f"RD{b}", name=f"RD{b}")
                 for b in range(2)]
        PS_ps = [ps.tile([P, NCOL], f32, tag=f"PS{b}", name=f"PS{b}")
                 for b in range(2)]

        ones_t = st.tile([P, 1], bf16, tag="ones")
        nc.vector.memset(ones_t[:], 1.0)

        NSTEPS = 2 * NCH
        state = {}

        def stage0(s):
            # DMA chunk s: [0.5*ls | prior] first (feeds ACT soonest)
            b, c = s % 2, s // 2
            t = io.tile([P, 4 * CHUNK], bf16, tag="pk", name=f"pk{s}")
            nc.sync.dma_start(
                t[:, 0:2 * CHUNK], ins[BRANCHES[b]][:, c, 0:2 * CHUNK])
            nc.sync.dma_start(
                t[:, 2 * CHUNK:4 * CHUNK],
                ins[BRANCHES[b]][:, c, 2 * CHUNK:4 * CHUNK])
            state[s] = t

        def stage1(s):
            # ACT: [std | pe] = exp([0.5*ls | prior]) in one instruction
            t = state[s]
            sp = pep.tile([P, 2 * CHUNK], bf16, tag="sp", name=f"sp{s}")
            nc.scalar.activation(sp[:], t[:, 0:2 * CHUNK], Exp)
            state[(s, "sp")] = sp

        def stage_pe_ps(s):
            # PE: 16 PS matmuls; depend only on sp(s), so they start the PE
            # stream early and fill it while DVE computes the chunk.
            b, c = s % 2, s // 2
            sp = state[(s, "sp")]
            for j in range(NBLK):
                col = c * NBLK + j
                nc.tensor.matmul(
                    PS_ps[b][:, col:col + 1],
                    sp[:, CHUNK + 128 * j:CHUNK + 128 * (j + 1)], ones_t[:],
                    start=True, stop=True)

        def stage2(s):
            # DVE se/enc (2x bf16), ACT e, DVE d
            t = state[s]
            sp = state[(s, "sp")]
            # se = std * eps            (into eps slot)
            nc.vector.tensor_tensor(
                t[:, 2 * CHUNK:3 * CHUNK], sp[:, 0:CHUNK],
                t[:, 2 * CHUNK:3 * CHUNK], mult)
            # enc = se + mean           (into mean slot)
            nc.vector.tensor_tensor(
                t[:, 3 * CHUNK:4 * CHUNK], t[:, 2 * CHUNK:3 * CHUNK],
                t[:, 3 * CHUNK:4 * CHUNK], add)
            ee = eep.tile([P, 2 * CHUNK], bf16, tag="ee", name=f"ee{s}")
            # e = exp(enc)
            nc.scalar.activation(ee[:, 0:CHUNK], t[:, 3 * CHUNK:4 * CHUNK], Exp)
            # d = enc - prior           (into dead ls slot; overlaps ACT e)
            nc.vector.tensor_tensor(
                t[:, 0:CHUNK], t[:, 3 * CHUNK:4 * CHUNK],
                t[:, CHUNK:2 * CHUNK], sub)
            state[(s, "ee")] = ee

        def stage2b(s):
            # ed = e * d, one round later so the in-order DVE stream never
            # blocks on the ACT queue (e(s) sits behind sp(s+1) there)
            t = state.pop(s)
            ee = state[(s, "ee")]
            nc.vector.tensor_tensor(
                ee[:, CHUNK:2 * CHUNK], ee[:, 0:CHUNK], t[:, 0:CHUNK], mult)

        def stage_pe_srd(s):
            # PE: 32 matmuls for S and RD of chunk s (e and ed blocks)
            b, c = s % 2, s // 2
            ee = state.pop((s, "ee"))
            state.pop((s, "sp"))
            for j in range(NBLK):
                col = c * NBLK + j
                nc.tensor.matmul(
                    S_ps[b][:, col:col + 1],
                    ee[:, 128 * j:128 * (j + 1)], ones_t[:],
                    start=True, stop=True)
            for j in range(NBLK):
                col = c * NBLK + j
                nc.tensor.matmul(
                    RD_ps[b][:, col:col + 1],
                    ee[:, CHUNK + 128 * j:CHUNK + 128 * (j + 1)], ones_t[:],
                    start=True, stop=True)

        def small_blocks():
            # cross entropy on y_zt: per-row lse - picked
            yoh_t = st.tile([P, 2 * YF], bf16, tag="yoh")
            nc.sync.dma_start(yoh_t[:], ins["yoh"][:])
            y_ap = yoh_t[:, 0:YF]
            oh_ap = yoh_t[:, YF:2 * YF]
            ey_t = st.tile([P, YF], bf16, tag="ey")
            nc.scalar.activation(ey_t[:], y_ap, Exp)
            sy_t = st.tile([P, NCOL], f32, tag="sy")
            nc.vector.tensor_reduce(
                sy_t[:], ey_t[:].rearrange("p (g c) -> p g c", c=C), X, add
            )
            lse_t = st.tile([P, NCOL], f32, tag="lse")
            nc.scalar.activation(lse_t[:], sy_t[:], Ln)
            ym_t = st.tile([P, YF], bf16, tag="ym")
            nc.vector.tensor_tensor(ym_t[:], y_ap, oh_ap, mult)
            pick_t = st.tile([P, NCOL], f32, tag="pick")
            nc.vector.tensor_reduce(
                pick_t[:], ym_t[:].rearrange("p (g c) -> p g c", c=C), X, add
            )
            nc.vector.tensor_tensor(
                out_sb[:, 2 * NCOL:3 * NCOL], lse_t[:], pick_t[:], sub
            )
            nc.sync.dma_start(
                out_d[:, 2 * NCOL:3 * NCOL], out_sb[:, 2 * NCOL:3 * NCOL])

            # entropy of softmax(s_zt): per-row lse - (sum e*x)/s
            sz_t = st.tile([P, SF], bf16, tag="sz")
            nc.sync.dma_start(sz_t[:], ins["sz"][:])
            esz_t = st.tile([P, SF], bf16, tag="esz")
            nc.scalar.activation(esz_t[:], sz_t[:], Exp)
            ssum_t = st.tile([P, NCOL], f32, tag="ssum")
            nc.vector.tensor_reduce(
                ssum_t[:], esz_t[:].rearrange("p (g c) -> p g c", c=S), X, add
            )
            exs_t = st.tile([P, SF], bf16, tag="exs")
            nc.vector.tensor_tensor(exs_t[:], esz_t[:], sz_t[:], mult)
            dsum_t = st.tile([P, NCOL], f32, tag="dsum")
            nc.vector.tensor_reduce(
                dsum_t[:], exs_t[:].rearrange("p (g c) -> p g c", c=S), X, add
            )
            rss_t = st.tile([P, NCOL], f32, tag="rss")
            nc.vector.reciprocal(rss_t[:], ssum_t[:])
            t2_t = st.tile([P, NCOL], f32, tag="t2")
            nc.vector.tensor_tensor(t2_t[:], dsum_t[:], rss_t[:], mult)
            lss_t = st.tile([P, NCOL], f32, tag="lss")
            nc.scalar.activation(lss_t[:], ssum_t[:], Ln)
            nc.vector.tensor_tensor(
                out_sb[:, 3 * NCOL:4 * NCOL], lss_t[:], t2_t[:], sub
            )
            nc.sync.dma_start(
                out_d[:, 3 * NCOL:4 * NCOL], out_sb[:, 3 * NCOL:4 * NCOL])

        # --- big-branch software pipeline ---
        # lags: DMA(i), sp(i-1), PE-PS(i-2), se/enc/e/d(i-2), ed(i-3),
        #       PE-S/RD(i-3). Small blocks emitted after round 1 so they
        #       don't delay sp(0) on the in-order ACT queue.
        for i in range(NSTEPS + 3):
            if i < NSTEPS:
                stage0(i)
            if 1 <= i and i - 1 < NSTEPS:
                stage1(i - 1)
            if i == 2:
                small_blocks()
            if 2 <= i and i - 2 < NSTEPS:
                stage_pe_ps(i - 2)
                stage2(i - 2)
            if 3 <= i and i - 3 < NSTEPS:
                stage2b(i - 3)
                stage_pe_srd(i - 3)

        # tails: kl_row = RD/S - ln S + ln PS  (once per branch)
        for b in range(2):
            rs_t = st.tile([P, NCOL], f32, tag=f"rs{b}", name=f"rs{b}")
            nc.vector.reciprocal(rs_t[:], S_ps[b][:])
            term_t = st.tile([P, NCOL], f32, tag=f"term{b}", name=f"term{b}")
            nc.vector.tensor_tensor(term_t[:], RD_ps[b][:], rs_t[:], mult)
            lnS_t = st.tile([P, NCOL], f32, tag=f"lnS{b}", name=f"lnS{b}")
            nc.scalar.activation(lnS_t[:], S_ps[b][:], Ln)
            lnPS_t = st.tile([P, NCOL], f32, tag=f"lnPS{b}", name=f"lnPS{b}")
            nc.scalar.activation(lnPS_t[:], PS_ps[b][:], Ln)
            tmp_t = st.tile([P, NCOL], f32, tag=f"tmp{b}", name=f"tmp{b}")
            nc.vector.tensor_tensor(tmp_t[:], term_t[:], lnS_t[:], sub)
            nc.vector.tensor_tensor(
                out_sb[:, b * NCOL:(b + 1) * NCOL], tmp_t[:], lnPS_t[:], add
            )
            nc.sync.dma_start(
                out_d[:, b * NCOL:(b + 1) * NCOL],
                out_sb[:, b * NCOL:(b + 1) * NCOL]
            )

    return nc


def _split_multi_waits(nc):
    """walrus's codegen allows a single embedded sync-wait per compute
    instruction; Tile sometimes emits two (e.g. ACT + DMA deps on one TT).
    Hoist all-but-one wait into standalone EventSemaphore instructions
    placed immediately before, on the same engine. Applied at BIR-JSON
    serialization time so CoreSim (which handles multi-wait fine) is
    untouched."""
    import json

    orig = nc.to_json_bytes

    def patched():
        bj = json.loads(orig())
        for fn in bj["functions"]:
            for blk in fn["blocks"]:
                new = []
                for inst in blk["instructions"]:
                    si = inst.get("sync_info") or {}
                    waits = si.get("on_wait") or []
                    if len(waits) > 1 and inst.get("opcode") != "EventSemaphore":
                        for i, w in enumerate(waits[:-1]):
                            new.append({
                                "debug": inst.get("debug"),
                                "engine": inst["engine"],
                                "ins": [],
                                "name": f"{inst['name']}-sw{i}",
                                "opcode": "EventSemaphore",
                                "outs": [],
                                "sync_info": {"on_update": [], "on_wait": [w]},
                            })
                        si["on_wait"] = [waits[-1]]
                    new.append(inst)
                blk["instructions"] = new
        return json.dumps(bj).encode()

    nc.to_json_bytes = patched
    return nc


def get_nc():
    global _CACHED_NC
    if _CACHED_NC is None:
        _CACHED_NC = _split_multi_waits(_build_nc())
    return _CACHED_NC


def make_in_maps(inputs):
    """Shard the full inputs into per-core in_maps for run_bass_kernel_spmd.

    Both branches D-on-partitions: [8192, 128].T -> [P, NCH, CHUNK] per
    slice, packed [P, NCH, 4, CHUNK] as [0.5*ls | prior | eps | mean].
    """
    import ml_dtypes
    bf16 = ml_dtypes.bfloat16
    f32 = np.float32
    arr = {k: np.asarray(v) for k, v in inputs.items()}
    target = np.asarray(arr["target"]).astype(np.int64).reshape(B)
    onehot = np.zeros((B, C), dtype=f32)
    onehot[np.arange(B), target] = 1.0

    big = {}
    for bn, srcs in (("bt", ("log_std_t", "eps_prior_t", "eps_t", "mean_t")),
                     ("bs", ("log_std_s", "eps_prior_s", "eps_s", "mean_s"))):
        mats = []
        for i, s in enumerate(srcs):
            a = np.asarray(arr[s], dtype=f32)
            if i == 0:
                a = a * 0.5
            mats.append(a.astype(bf16))
        big[bn] = mats

    in_maps = []
    for cidx in range(NCORES):
        sl = slice(cidx * RPC, (cidx + 1) * RPC)
        m = {}
        for bn in BRANCHES:
            pk = np.stack(
                [np.ascontiguousarray(a[sl].T).reshape(P, NCH, CHUNK)
                 for a in big[bn]], axis=2)
            m[bn] = np.ascontiguousarray(pk.reshape(P, NCH, 4 * CHUNK))
        yoh = np.empty((P, 2 * YF), dtype=bf16)
        yoh[:, :YF] = np.ascontiguousarray(arr["y_zt"][sl], dtype=f32).reshape(P, YF).astype(bf16)
        yoh[:, YF:] = np.ascontiguousarray(onehot[sl]).reshape(P, YF).astype(bf16)
        m["yoh"] = yoh
        m["sz"] = np.ascontiguousarray(arr["s_zt"][sl], dtype=f32).reshape(P, SF).astype(bf16)
        in_maps.append(m)
    return in_maps


def combine(outs, current_step):
    """Host-side unshard: f64 reduce of per-row partials -> final f32 scalar."""
    tot = np.zeros(4, dtype=np.float64)
    for o in outs:
        o = o.reshape(P, 4, NCOL)
        tot += o.sum(axis=(0, 2), dtype=np.float64)
    L_zt, L_zs, L_t, Loss_e = tot / B
    frac = float(current_step) / STEP_SIZE
    lam_e = LAMBDA_E * GAMMA_E ** frac
    lam_od = LAMBDA_OD * GAMMA_OD ** frac
    val = L_t + lam_e * Loss_e + lam_od * (L_zt + L_zs)
    return np.array(val, dtype=np.float32)


def _install_ntff_hook():
    """Best-effort: register the axon NTFF profiling hook that the agent
    image's antenv package is missing, so trace=True yields exec_time_ns."""
    try:
        import sys, types
        import antenv
        if "antenv.axon_hooks" in sys.modules:
            return True
        sys.path.insert(0, "/root/.axon_site/trn_agent_boot")
        import trn_boot
        mod = types.ModuleType("antenv.axon_hooks")
        _h = {}
        mod.set_axon_ntff_profile_hook = lambda h: _h.__setitem__("h", h)
        mod.get_axon_ntff_profile_hook = lambda: _h.get("h")
        sys.modules["antenv.axon_hooks"] = mod
        antenv.axon_hooks = mod
        mod.set_axon_ntff_profile_hook(
            trn_boot._ntff_profile_via_ctypes("/opt/axon/libaxon_pjrt.so")
        )
        import concourse.bass_utils as bu
        bu.upload_artifacts = lambda tmpdir: str(tmpdir)
        return True
    except Exception:
        return False


def kernel(**inputs):
    global LAST_EXEC_NS
    from concourse.bass_utils import run_bass_kernel_spmd

    trace = os.environ.get("BASS_KERNEL_TRACE", "0") == "1"
    if trace:
        trace = _install_ntff_hook()

    nc = get_nc()
    in_maps = make_in_maps(inputs)
    res = run_bass_kernel_spmd(
        nc, in_maps, list(range(NCORES)), trace=trace
    )
    LAST_EXEC_NS = res.exec_time_ns
    outs = [r["out"] for r in res.results]
    cs = inputs.get("current_step", 500)
    return combine(outs, int(np.asarray(cs)))


# revision 11
# speedup vs baseline: 1.3257x; 1.1032x over previous
"""Trainium2 Bass kernel for nn_Criterion_37984690765901.

Loss =  L_t + lam_e * Loss_e + lam_od * (L_zt + L_zs)
  L_t    = mean_r( lse(y_zt_r) - y_zt[r, target_r] )            (cross entropy)
  Loss_e = mean_r( lse(s_r) - (sum_j e^{s_rj} s_rj)/sum_j e^{s_rj} )   (entropy)
  L_zt/L_zs = mean_r( rowdot_r/S_r - ln S_r + ln PS_r )          (KLD batchmean)
     with enc = mean + exp(0.5*log_std)*eps,  e = exp(enc), S = sum_d e,
     pe = exp(prior), PS = sum_d pe, rowdot = sum_d e*(enc - prior).
     (prior_s = 1 + eps_prior_s, but KLD is shift-invariant in the prior
      logits, so eps_prior_s is used directly.)

v5 design (baseline f32 151.6us; v2 all-PE 75.5us; hybrid DVE-tree 93.8us):
  * All big tensors shipped bf16 (halves HBM traffic; host sim of the full
    quantization chain gives rel err ~4e-6 vs the 2e-2 gate). log_std is
    pre-halved on the host so [0.5*ls | prior] exps in one ACT op.
  * D=128 on partitions, rows on the free axis. Elementwise chain on DVE at
    2x (16-bit packed) + ACT exp; all six per-row sum_d reductions are
    TensorEngine data-as-stationary x ones[128,1] matmuls into PSUM columns.
    Measured: one [128x128] block costs ~140ns effective (LDW overlaps MM),
    giving a dense ~54us PE stream — the critical path, so everything else
    is arranged to keep PE fed from t~8us with zero stalls:
      - PS matmuls (need only sp = exp([ls'|prior])) run at pipeline lag 2,
        S/RD matmuls at lag 3, so the PE stream starts as soon as the first
        chunk's exps land and never waits mid-stream (DVE+ACT per round are
        faster than PE's 6.7us).
      - the small y_zt/s_zt blocks are emitted after the first big round so
        they don't delay sp(0) on the in-order ACT queue.
      - ed lags one round on DVE so the in-order DVE stream never blocks on
        the ACT queue (e(s) sits behind sp(s+1) there).

Sharding: pure data parallel over the batch axis, 8192 rows per core; the
batch reduction finishes on the host in float64.

Device per-core outputs: out[128, 256] f32 =
  [:, 0:64]    per-row KL contribution, t branch   (row = col*128 + p)
  [:, 64:128]  per-row KL contribution, s branch   (row = col*128 + p)
  [:, 128:192] per-row (lse_y - y_pick)            (row = 64p + j)
  [:, 192:256] per-row entropy of softmax(s_zt)
Host combine just sums everything in f64, so orderings don't matter.
"""

import os
import numpy as np

NCORES = 8
B, D, C, S = 65536, 128, 10, 2
LAMBDA_E, LAMBDA_OD = 0.1, 0.036
GAMMA_E, GAMMA_OD = 2.0, 2.0
STEP_SIZE = 1000.0

RPC = B // NCORES            # rows per core = 8192
P = 128                      # SBUF partitions
CHUNK = 1024                 # rows per chunk
NCH = RPC // CHUNK           # 4 chunks per tensor
NBLK = CHUNK // 128          # 16 row-blocks (matmuls) per chunk per stat
NCOL = 64                    # stat columns per branch (8192/128)
YF = RPC * C // P            # 640
SF = RPC * S // P            # 128

BRANCHES = ["bt", "bs"]

_CACHED_NC = None
LAST_EXEC_NS = None


def _build_nc():
    import concourse.bass as bass
    import concourse.tile as tile
    from concourse import mybir
    from contextlib import ExitStack

    f32 = mybir.dt.float32
    bf16 = mybir.dt.bfloat16
    Exp = mybir.ActivationFunctionType.Exp
    Ln = mybir.ActivationFunctionType.Ln
    add = mybir.AluOpType.add
    sub = mybir.AluOpType.subtract
    mult = mybir.AluOpType.mult
    X = mybir.AxisListType.X

    nc = bass.Bass("TRN2", debug=False)

    ins = {}
    for bn in BRANCHES:
        ins[bn] = nc.dram_tensor(
            bn, [P, NCH, 4 * CHUNK], bf16, kind="ExternalInput"
        ).ap()
    ins["yoh"] = nc.dram_tensor("yoh", [P, 2 * YF], bf16, kind="ExternalInput").ap()
    ins["sz"] = nc.dram_tensor("sz", [P, SF], bf16, kind="ExternalInput").ap()
    out_d = nc.dram_tensor("out", [P, 4 * NCOL], f32, kind="ExternalOutput").ap()

    with tile.TileContext(nc) as tc, ExitStack() as ctx:
        io = ctx.enter_context(tc.tile_pool(name="io", bufs=5))
        pep = ctx.enter_context(tc.tile_pool(name="pep", bufs=4))
        eep = ctx.enter_context(tc.tile_pool(name="eep", bufs=4))
        st = ctx.enter_context(tc.tile_pool(name="st", bufs=1))
        ps = ctx.enter_context(tc.psum_pool(name="ps", bufs=1))

        out_sb = st.tile([P, 4 * NCOL], f32, tag="out")

        # PSUM stat tiles: column col = chunk*NBLK + blk holds rows
        # col*128 .. col*128+127 of this core's branch shard.
        S_ps = [ps.tile([P, NCOL], f32, tag=f"S{b}", name=f"S{b}")
                for b in range(2)]
        RD_ps = [ps.tile([P, NCOL], f32, tag=f"RD{b}", name=f"RD{b}")
                 for b in range(2)]
        PS_ps = [ps.tile([P, NCOL], f32, tag=f"PS{b}", name=f"PS{b}")
                 for b in range(2)]

        ones_t = st.tile([P, 1], bf16, tag="ones")
        nc.vector.memset(ones_t[:], 1.0)

        NSTEPS = 2 * NCH
        state = {}

        def stage0(s):
            # DMA chunk s: [0.5*ls | prior] first (feeds ACT soonest)
            b, c = s % 2, s // 2
            t = io.tile([P, 4 * CHUNK], bf16, tag="pk", name=f"pk{s}")
            nc.sync.dma_start(
                t[:, 0:2 * CHUNK], ins[BRANCHES[b]][:, c, 0:2 * CHUNK])
            nc.sync.dma_start(
                t[:, 2 * CHUNK:4 * CHUNK],
                ins[BRANCHES[b]][:, c, 2 * CHUNK:4 * CHUNK])
            state[s] = t

        def stage1(s):
            # ACT: [std | pe] = exp([0.5*ls | prior]) in one instruction
            t = state[s]
            sp = pep.tile([P, 2 * CHUNK], bf16, tag="sp", name=f"sp{s}")
            nc.scalar.activation(sp[:], t[:, 0:2 * CHUNK], Exp)
            state[(s, "sp")] = sp

        def stage_pe_ps(s):
            # PE: 16 PS matmuls; depend only on sp(s), so they start the PE
            # stream early and fill it while DVE computes the chunk.
            b, c = s % 2, s // 2
            sp = state[(s, "sp")]
            for j in range(NBLK):
                col = c * NBLK + j
                nc.tensor.matmul(
                    PS_ps[b][:, col:col + 1],
                    sp[:, CHUNK + 128 * j:CHUNK + 128 * (j + 1)], ones_t[:],
                    start=True, stop=True)

        def stage2(s):
            # DVE se/enc (2x bf16), ACT e, DVE d
            t = state[s]
            sp = state[(s, "sp")]
            # se = std * eps            (into eps slot)
            nc.vector.tensor_tensor(
                t[:, 2 * CHUNK:3 * CHUNK], sp[:, 0:CHUNK],
                t[:, 2 * CHUNK:3 * CHUNK], mult)
            # enc = se + mean           (into mean slot)
            nc.vector.tensor_tensor(
                t[:, 3 * CHUNK:4 * CHUNK], t[:, 2 * CHUNK:3 * CHUNK],
                t[:, 3 * CHUNK:4 * CHUNK], add)
            ee = eep.tile([P, 2 * CHUNK], bf16, tag="ee", name=f"ee{s}")
            # e = exp(enc)
            nc.scalar.activation(ee[:, 0:CHUNK], t[:, 3 * CHUNK:4 * CHUNK], Exp)
            # d = enc - prior           (into dead ls slot; overlaps ACT e)
            nc.vector.tensor_tensor(
                t[:, 0:CHUNK], t[:, 3 * CHUNK:4 * CHUNK],
                t[:, CHUNK:2 * CHUNK], sub)
            state[(s, "ee")] = ee

        def stage2b(s):
            # ed = e * d, one round later so the in-order DVE stream never
            # blocks on the ACT queue (e(s) sits behind sp(s+1) there)
            t = state.pop(s)
            ee = state[(s, "ee")]
            nc.vector.tensor_tensor(
                ee[:, CHUNK:2 * CHUNK], ee[:, 0:CHUNK], t[:, 0:CHUNK], mult)

        def stage_pe_srd(s):
            # PE: 32 matmuls for S and RD of chunk s (e and ed blocks)
            b, c = s % 2, s // 2
            ee = state.pop((s, "ee"))
            state.pop((s, "sp"))
            for j in range(NBLK):
                col = c * NBLK + j
                nc.tensor.matmul(
                    S_ps[b][:, col:col + 1],
                    ee[:, 128 * j:128 * (j + 1)], ones_t[:],
                    start=True, stop=True)
            for j in range(NBLK):
                col = c * NBLK + j
                nc.tensor.matmul(
                    RD_ps[b][:, col:col + 1],
                    ee[:, CHUNK + 128 * j:CHUNK + 128 * (j + 1)], ones_t[:],
                    start=True, stop=True)

        def small_blocks():
            # cross entropy on y_zt: per-row lse - picked
            yoh_t = st.tile([P, 2 * YF], bf16, tag="yoh")
            nc.sync.dma_start(yoh_t[:], ins["yoh"][:])
            y_ap = yoh_t[:, 0:YF]
            oh_ap = yoh_t[:, YF:2 * YF]
            ey_t = st.tile([P, YF], bf16, tag="ey")
            nc.scalar.activation(ey_t[:], y_ap, Exp)
            sy_t = st.tile([P, NCOL], f32, tag="sy")
            nc.vector.tensor_reduce(
                sy_t[:], ey_t[:].rearrange("p (g c) -> p g c", c=C), X, add
            )
            lse_t = st.tile([P, NCOL], f32, tag="lse")
            nc.scalar.activation(lse_t[:], sy_t[:], Ln)
            ym_t = st.tile([P, YF], bf16, tag="ym")
            nc.vector.tensor_tensor(ym_t[:], y_ap, oh_ap, mult)
            pick_t = st.tile([P, NCOL], f32, tag="pick")
            nc.vector.tensor_reduce(
                pick_t[:], ym_t[:].rearrange("p (g c) -> p g c", c=C), X, add
            )
            nc.vector.tensor_tensor(
                out_sb[:, 2 * NCOL:3 * NCOL], lse_t[:], pick_t[:], sub
            )
            nc.sync.dma_start(
                out_d[:, 2 * NCOL:3 * NCOL], out_sb[:, 2 * NCOL:3 * NCOL])

            # entropy of softmax(s_zt): per-row lse - (sum e*x)/s
            sz_t = st.tile([P, SF], bf16, tag="sz")
            nc.sync.dma_start(sz_t[:], ins["sz"][:])
            esz_t = st.tile([P, SF], bf16, tag="esz")
            nc.scalar.activation(esz_t[:], sz_t[:], Exp)
            ssum_t = st.tile([P, NCOL], f32, tag="ssum")
            nc.vector.tensor_reduce(
                ssum_t[:], esz_t[:].rearrange("p (g c) -> p g c", c=S), X, add
            )
            exs_t = st.tile([P, SF], bf16, tag="exs")
            nc.vector.tensor_tensor(exs_t[:], esz_t[:], sz_t[:], mult)
            dsum_t = st.tile([P, NCOL], f32, tag="dsum")
            nc.vector.tensor_reduce(
                dsum_t[:], exs_t[:].rearrange("p (g c) -> p g c", c=S), X, add
            )
            rss_t = st.tile([P, NCOL], f32, tag="rss")
            nc.vector.reciprocal(rss_t[:], ssum_t[:])
            t2_t = st.tile([P, NCOL], f32, tag="t2")
            nc.vector.tensor_tensor(t2_t[:], dsum_t[:], rss_t[:], mult)
            lss_t = st.tile([P, NCOL], f32, tag="lss")
            nc.scalar.activation(lss_t[:], ssum_t[:], Ln)
            nc.vector.tensor_tensor(
                out_sb[:, 3 * NCOL:4 * NCOL], lss_t[:], t2_t[:], sub
            )
            nc.sync.dma_start(
                out_d[:, 3 * NCOL:4 * NCOL], out_sb[:, 3 * NCOL:4 * NCOL])

        # --- big-branch software pipeline ---
        # lags: DMA(i), sp(i-1), PE-PS(i-2), se/enc/e/d(i-2), ed(i-3),
        #       PE-S/RD(i-3). Small blocks emitted after round 1 so they
        #       don't delay sp(0) on the in-order ACT queue.
        for i in range(NSTEPS + 3):
            if i < NSTEPS:
                stage0(i)
            if 1 <= i and i - 1 < NSTEPS:
                stage1(i - 1)
            if i == 2:
                small_blocks()
            if 2 <= i and i - 2 < NSTEPS:
                stage_pe_ps(i - 2)
                stage2(i - 2)
            if 3 <= i and i - 3 < NSTEPS:
                stage2b(i - 3)
                stage_pe_srd(i - 3)

        # tails: kl_row = RD/S - ln S + ln PS  (once per branch)
        for b in range(2):
            rs_t = st.tile([P, NCOL], f32, tag=f"rs{b}", name=f"rs{b}")
            nc.vector.reciprocal(rs_t[:], S_ps[b][:])
            term_t = st.tile([P, NCOL], f32, tag=f"term{b}", name=f"term{b}")
            nc.vector.tensor_tensor(term_t[:], RD_ps[b][:], rs_t[:], mult)
            lnS_t = st.tile([P, NCOL], f32, tag=f"lnS{b}", name=f"lnS{b}")
            nc.scalar.activation(lnS_t[:], S_ps[b][:], Ln)
            lnPS_t = st.tile([P, NCOL], f32, tag=f"lnPS{b}", name=f"lnPS{b}")
            nc.scalar.activation(lnPS_t[:], PS_ps[b][:], Ln)
            tmp_t = st.tile([P, NCOL], f32, tag=f"tmp{b}", name=f"tmp{b}")
            nc.vector.tensor_tensor(tmp_t[:], term_t[:], lnS_t[:], sub)
            nc.vector.tensor_tensor(
                out_sb[:, b * NCOL:(b + 1) * NCOL], tmp_t[:], lnPS_t[:], add
            )
            nc.sync.dma_start(
                out_d[:, b * NCOL:(b + 1) * NCOL],
                out_sb[:, b * NCOL:(b + 1) * NCOL]
            )

    return nc


def _split_multi_waits(nc):
    """walrus's codegen allows a single embedded sync-wait per compute
    instruction; Tile sometimes emits two (e.g. ACT + DMA deps on one TT).
    Hoist all-but-one wait into standalone EventSemaphore instructions
    placed immediately before, on the same engine. Applied at BIR-JSON
    serialization time so CoreSim (which handles multi-wait fine) is
    untouched."""
    import json

    orig = nc.to_json_bytes

    def patched():
        bj = json.loads(orig())
        for fn in bj["functions"]:
            for blk in fn["blocks"]:
                new = []
                for inst in blk["instructions"]:
                    si = inst.get("sync_info") or {}
                    waits = si.get("on_wait") or []
                    if len(waits) > 1 and inst.get("opcode") != "EventSemaphore":
                        for i, w in enumerate(waits[:-1]):
                            new.append({
                                "debug": inst.get("debug"),
                                "engine": inst["engine"],
                                "ins": [],
                                "name": f"{inst['name']}-sw{i}",
                                "opcode": "EventSemaphore",
                                "outs": [],
                                "sync_info": {"on_update": [], "on_wait": [w]},
                            })
                        si["on_wait"] = [waits[-1]]
                    new.append(inst)
                blk["instructions"] = new
        return json.dumps(bj).encode()

    nc.to_json_bytes = patched
    return nc


def get_nc():
    global _CACHED_NC
    if _CACHED_NC is None:
        _CACHED_NC = _split_multi_waits(_build_nc())
    return _CACHED_NC


def make_in_maps(inputs):
    """Shard the full inputs into per-core in_maps for run_bass_kernel_spmd.

    Both branches D-on-partitions: [8192, 128].T -> [P, NCH, CHUNK] per
    slice, packed [P, NCH, 4, CHUNK] as [0.5*ls | prior | eps | mean].
    """
    import ml_dtypes
    bf16 = ml_dtypes.bfloat16
    f32 = np.float32
    arr = {k: np.asarray(v) for k, v in inputs.items()}
    target = np.asarray(arr["target"]).astype(np.int64).reshape(B)
    onehot = np.zeros((B, C), dtype=f32)
    onehot[np.arange(B), target] = 1.0

    big = {}
    for bn, srcs in (("bt", ("log_std_t", "eps_prior_t", "eps_t", "mean_t")),
                     ("bs", ("log_std_s", "eps_prior_s", "eps_s", "mean_s"))):
        mats = []
        for i, s in enumerate(srcs):
            a = np.asarray(arr[s], dtype=f32)
            if i == 0:
                a = a * 0.5
            mats.append(a.astype(bf16))
        big[bn] = mats

    in_maps = []
    for cidx in range(NCORES):
        sl = slice(cidx * RPC, (cidx + 1) * RPC)
        m = {}
        for bn in BRANCHES:
            pk = np.stack(
                [np.ascontiguousarray(a[sl].T).reshape(P, NCH, CHUNK)
                 for a in big[bn]], axis=2)
            m[bn] = np.ascontiguousarray(pk.reshape(P, NCH, 4 * CHUNK))
        yoh = np.empty((P, 2 * YF), dtype=bf16)
        yoh[:, :YF] = np.ascontiguousarray(arr["y_zt"][sl], dtype=f32).reshape(P, YF).astype(bf16)
        yoh[:, YF:] = np.ascontiguousarray(onehot[sl]).reshape(P, YF).astype(bf16)
        m["yoh"] = yoh
        m["sz"] = np.ascontiguousarray(arr["s_zt"][sl], dtype=f32).reshape(P, SF).astype(bf16)
        in_maps.append(m)
    return in_maps


def combine(outs, current_step):
    """Host-side unshard: f64 reduce of per-row partials -> final f32 scalar."""
    tot = np.zeros(4, dtype=np.float64)
    for o in outs:
        o = o.reshape(P, 4, NCOL)
        tot += o.sum(axis=(0, 2), dtype=np.float64)
    L_zt, L_zs, L_t, Loss_e = tot / B
    frac = float(current_step) / STEP_SIZE
    lam_e = LAMBDA_E * GAMMA_E ** frac
    lam_od = LAMBDA_OD * GAMMA_OD ** frac
    val = L_t + lam_e * Loss_e + lam_od * (L_zt + L_zs)
    return np.array(val, dtype=np.float32)


def _install_ntff_hook():
    """Best-effort: register the axon NTFF profiling hook that the agent
    image's antenv package is missing, so trace=True yields exec_time_ns."""
    try:
        import sys, types
        import antenv
        if "antenv.axon_hooks" in sys.modules:
            return True
        sys.path.insert(0, "/root/.axon_site/trn_agent_boot")
        import trn_boot
        mod = types.ModuleType("antenv.axon_hooks")
        _h = {}
        mod.set_axon_ntff_profile_hook = lambda h: _h.__setitem__("h", h)
        mod.get_axon_ntff_profile_hook = lambda: _h.get("h")
        sys.modules["antenv.axon_hooks"] = mod
        antenv.axon_hooks = mod
        mod.set_axon_ntff_profile_hook(
            trn_boot._ntff_profile_via_ctypes("/opt/axon/libaxon_pjrt.so")
        )
        import concourse.bass_utils as bu
        bu.upload_artifacts = lambda tmpdir: str(tmpdir)
        return True
    except Exception:
        return False


def kernel(**inputs):
    global LAST_EXEC_NS
    from concourse.bass_utils import run_bass_kernel_spmd

    trace = os.environ.get("BASS_KERNEL_TRACE", "0") == "1"
    if trace:
        trace = _install_ntff_hook()

    nc = get_nc()
    in_maps = make_in_maps(inputs)
    res = run_bass_kernel_spmd(
        nc, in_maps, list(range(NCORES)), trace=trace
    )
    LAST_EXEC_NS = res.exec_time_ns
    outs = [r["out"] for r in res.results]
    cs = inputs.get("current_step", 500)
    return combine(outs, int(np.asarray(cs)))
